# revision 21
# baseline (speedup 1.0000x reference)
"""Trainium2 Bass kernel for AttentionWithCAE.

Reference computation (B=8, N=1024, C=768, H=12, hd=64):
    qkv  = x @ qkv_w.T + concat(q_bias, 0, v_bias)
    q,k,v per head; attn = softmax(mask(q*scale @ k.T)); out = attn @ v
    final = out @ proj_w.T + proj_b

Sharding: pure data parallel — batch b on core b, weights replicated,
no collectives.

Device-side layout strategy (per core):
  - Host pre-transposes operands so the device kernel does zero transposes:
      xT [C, N], wqkT [C, 3C] (q-cols pre-scaled by SCALE), pwT [C, C],
      all cast to bf16 on the host (PSUM accumulation stays fp32).
  - qk projection emitted as qkT [1536, N] (feature-major): head h's qT/kT
    are rows h*64..h*64+64 — exactly the lhsT/rhs layout the scores matmul
    needs (contraction over head_dim).
  - v projection emitted token-major [N, 768] interleaved into v65 tiles
    [128, 12*65]: per head 64 v-columns plus a baked ones column, so one
    M=65 matmul per (head, k-tile, q-chunk) yields both attn@v and the
    softmax denominators (row 64 of PSUM).
  - scores computed transposed [k, q]: the key-dependent mask bias becomes a
    per-partition bias folded into the Exp activation (single ACT op;
    no max-subtraction needed: |scores| <= ~10 so exp can't overflow).
  - softmax denominators -> SBUF -> approx reciprocal -> partition-broadcast
    via a DRAM bounce (DMA broadcast needs a DRAM source).
  - attn output accumulates transposed [hd, t] which directly feeds the
    proj matmul; final output is [C, N] and the host transposes it back.
  - q_bias folds into the qkT eviction (per-partition bias); v_bias folds
    into an effective proj bias on the host (attn rows sum to 1).

Scheduling (the emission order shapes the per-engine execution order):
  - v-projection first, then per head-pair p: its two qkT tiles, then the
    pair's scores (row-packed: even head rows 0-63, odd head rows 64-127 ->
    concurrent K=64 matmuls), with the PREVIOUS pair's attn@v matmuls
    interleaved kt-by-kt. QKV work for pair p+1 fills PE gaps while ACT
    runs the exps of pair p, keeping the PE dense (no HAM re-throttle).
"""

import sys

sys.path.insert(0, "/opt/trn_rl_repo")

from contextlib import ExitStack

import numpy as np
import ml_dtypes

import concourse.bass as bass
import concourse.bacc as bacc
import concourse.mybir as mybir
from concourse import tile
from concourse.bass_utils import run_bass_kernel_spmd

B, N, C = 8, 1024, 768
H, HD = 12, 64
F3 = 3 * C  # 2304
SCALE = HD ** -0.5
F32 = mybir.dt.float32
BF16 = mybir.dt.bfloat16
Act = mybir.ActivationFunctionType

MASK_NEG = -30000.0

CT = C // 128  # 6 contraction tiles
TT = N // 128  # 8 token tiles
QKT = 2 * C // 128  # 12 qk feature tiles
NPAIR = H // 2  # 6 head pairs

_CACHE = {}


def _build_nc():
    nc = bacc.Bacc(None, target_bir_lowering=False)

    xT_d = nc.declare_dram_parameter("xT", [C, N], BF16, isOutput=False)
    wqk_d = nc.declare_dram_parameter("wqkT", [C, F3], BF16, isOutput=False)
    pw_d = nc.declare_dram_parameter("pwT", [C, C], BF16, isOutput=False)
    qkb_d = nc.declare_dram_parameter("qkb", [2 * C], F32, isOutput=False)
    mb_d = nc.declare_dram_parameter("mb", [N], F32, isOutput=False)
    pb_d = nc.declare_dram_parameter("pb", [C], F32, isOutput=False)
    out_d = nc.declare_dram_parameter("out", [C, N], F32, isOutput=True)

    r_d = nc.dram_tensor("r_scratch", [H, N], F32)

    with ExitStack() as ctx:
        tc = ctx.enter_context(tile.TileContext(nc))
        pool = ctx.enter_context(tc.tile_pool(name="main", bufs=1))
        psum = ctx.enter_context(tc.tile_pool(name="psum", bufs=1, space="PSUM"))

        F32R = mybir.dt.float32r
        ones64 = pool.tile([1, 64], F32R, tag="ones64", bufs=1)
        nc.vector.memset(ones64.bitcast(F32), 1.0)
        qkb_sb = pool.tile([128, QKT], F32)
        nc.sync.dma_start(out=qkb_sb, in_=qkb_d.rearrange("(i p) -> p i", p=128))
        mb_sb = pool.tile([128, TT], F32)
        nc.sync.dma_start(out=mb_sb, in_=mb_d.rearrange("(i p) -> p i", p=128))
        pb_sb = pool.tile([128, CT], F32)
        nc.sync.dma_start(out=pb_sb, in_=pb_d.rearrange("(i p) -> p i", p=128))

        wqk = []
        xTs = []
        for c in range(CT):
            w = pool.tile([128, F3], BF16, tag="wqk", bufs=CT, name=f"wqk{c}")
            for j in range(4):
                nc.sync.dma_start(
                    out=w[:, j * 576 : (j + 1) * 576],
                    in_=wqk_d[c * 128 : (c + 1) * 128, j * 576 : (j + 1) * 576],
                )
            wqk.append(w)
            xt = pool.tile([128, N], BF16, tag="xT", bufs=CT, name=f"xT{c}")
            for j in range(2):
                nc.sync.dma_start(
                    out=xt[:, j * 512 : (j + 1) * 512],
                    in_=xT_d[c * 128 : (c + 1) * 128, j * 512 : (j + 1) * 512],
                )
            xTs.append(xt)
        pw = []
        for c in range(CT):
            w = pool.tile([128, C], BF16, tag="pw", bufs=CT, name=f"pw{c}")
            nc.sync.dma_start(out=w, in_=pw_d[c * 128 : (c + 1) * 128, :])
            pw.append(w)

        qkT = [
            pool.tile([128, N], BF16, tag="qkT", bufs=QKT, name=f"qkT{i}")
            for i in range(QKT)
        ]
        v65 = [
            pool.tile([128, H * 65], BF16, tag="v65", bufs=TT, name=f"v65_{i}")
            for i in range(TT)
        ]
        aoT = [
            pool.tile([128, N], BF16, tag="aoT", bufs=CT, name=f"aoT{i}")
            for i in range(CT)
        ]

        # ---------------- v projection (needed by every head's AV) --------
        for ti in range(TT):
            psa = psum.tile([128, 512], F32, tag=f"psAV{ti % 2}0", bufs=1, name=f"ps_va{ti}")
            psb = psum.tile([128, 256], F32, tag=f"psAV{ti % 2}1", bufs=1, name=f"ps_vb{ti}")
            for c in range(CT):
                nc.tensor.matmul(
                    psa,
                    lhsT=xTs[c][:, ti * 128 : (ti + 1) * 128],
                    rhs=wqk[c][:, 1536:2048],
                    start=(c == 0),
                    stop=(c == CT - 1),
                )
                nc.tensor.matmul(
                    psb,
                    lhsT=xTs[c][:, ti * 128 : (ti + 1) * 128],
                    rhs=wqk[c][:, 2048:2304],
                    start=(c == 0),
                    stop=(c == CT - 1),
                )
            v3 = v65[ti].rearrange("p (h j) -> p h j", j=65)
            nc.scalar.activation(
                v3[:, 0:8, 0:64], psa.rearrange("p (h j) -> p h j", j=64), Act.Copy
            )
            nc.scalar.activation(
                v3[:, 8:12, 0:64], psb.rearrange("p (h j) -> p h j", j=64), Act.Copy
            )
            nc.vector.memset(v3[:, :, 64:65], 1.0)

        def emit_qk_tile(fi):
            ps = psum.tile([128, N], F32, tag="psA", bufs=2, name=f"ps_qk{fi}")
            for c in range(CT):
                for qc in range(2):
                    nc.tensor.matmul(
                        ps[:, qc * 512 : (qc + 1) * 512],
                        lhsT=wqk[c][:, fi * 128 : (fi + 1) * 128],
                        rhs=xTs[c][:, qc * 512 : (qc + 1) * 512],
                        start=(c == 0),
                        stop=(c == CT - 1),
                    )
            nc.vector.tensor_scalar_add(
                out=qkT[fi], in0=ps, scalar1=qkb_sb[:, fi : fi + 1]
            )

        def emit_av_kt(pr, kt):
            for hi, (h, atiles) in enumerate(
                [(pr["h0"], pr["at0"]), (pr["h1"], pr["at1"])]
            ):
                for qc in range(2):
                    nc.tensor.matmul(
                        pr["pav"][hi][qc][0:65, :],
                        lhsT=v65[kt][:, h * 65 : (h + 1) * 65],
                        rhs=atiles[kt][:, qc * 512 : (qc + 1) * 512],
                        start=(kt == 0),
                        stop=(kt == TT - 1),
                    )

        def finish_pair(pr):
            # Evict AV PSUM to SBUF right away (fast DVE copies release the
            # PSUM banks so the next pair's AV can start), then run the slow
            # normalization chain (recip -> DRAM-bounce broadcast -> mul)
            # entirely from SBUF.
            for hi, h in enumerate([pr["h0"], pr["h1"]]):
                qt, row = h // 2, (h % 2) * 64
                pav = pr["pav"][hi]
                un = [
                    pool.tile([64, 512], F32, tag=f"un{qc}", bufs=2, name=f"un{h}_{qc}")
                    for qc in range(2)
                ]
                srow = pool.tile([1, N], F32, tag="srow", bufs=2, name=f"s{h}")
                for qc in range(2):
                    nc.vector.tensor_copy(out=un[qc], in_=pav[qc][0:64, :])
                    nc.vector.tensor_copy(
                        out=srow[:, qc * 512 : (qc + 1) * 512], in_=pav[qc][64:65, :]
                    )
                r_row = pool.tile([1, N], F32R, tag="rrow", bufs=2, name=f"r{h}")
                r_f32 = pool.tile([1, N], F32, tag="rf", bufs=2, name=f"rf{h}")
                nc.vector.reciprocal_approx_fast(out=r_f32, in_=srow)
                nc.vector.tensor_copy(out=r_row, in_=r_f32)
                r2ps = psum.tile([64, N], F32, tag="psA", bufs=2, name=f"r2p{h}")
                for qc in range(2):
                    nc.tensor.matmul(
                        r2ps[:, qc * 512 : (qc + 1) * 512],
                        lhsT=ones64,
                        rhs=r_row[:, qc * 512 : (qc + 1) * 512],
                        start=True,
                        stop=True,
                    )
                for qc in range(2):
                    nc.vector.tensor_mul(
                        out=aoT[qt][row : row + 64, qc * 512 : (qc + 1) * 512],
                        in0=un[qc][0:64, :],
                        in1=r2ps[0:64, qc * 512 : (qc + 1) * 512],
                    )

        for p in range(NPAIR):
            emit_qk_tile(p)
            emit_qk_tile(CT + p)
            h0, h1 = 2 * p, 2 * p + 1
            k_tile, q_tile = qkT[CT + p], qkT[p]
            at0, at1 = [], []
            pav = []
            for hi in range(2):
                row = [
                    psum.tile(
                        [128, 512],
                        F32,
                        tag=f"psAV{hi}{qc}",
                        bufs=1,
                        name=f"pav{2 * p + hi}_{qc}",
                    )
                    for qc in range(2)
                ]
                pav.append(row)
            cur = {"h0": h0, "h1": h1, "at0": at0, "at1": at1, "pav": pav}
            for kt in range(TT):
                ps0 = psum.tile([128, N], F32, tag="psA", bufs=2, name=f"ps_s{h0}_{kt}")
                ps1 = psum.tile([128, N], F32, tag="psA", bufs=2, name=f"ps_s{h1}_{kt}")
                for qc in range(2):
                    # row-packed pair: even head rows 0-63, odd head rows 64-127
                    nc.tensor.matmul(
                        ps0[:, qc * 512 : (qc + 1) * 512],
                        lhsT=k_tile[0:64, kt * 128 : (kt + 1) * 128],
                        rhs=q_tile[0:64, qc * 512 : (qc + 1) * 512],
                        start=True,
                        stop=True,
                    )
                    nc.tensor.matmul(
                        ps1[:, qc * 512 : (qc + 1) * 512],
                        lhsT=k_tile[64:128, kt * 128 : (kt + 1) * 128],
                        rhs=q_tile[64:128, qc * 512 : (qc + 1) * 512],
                        start=True,
                        stop=True,
                    )
                if kt >= 2:
                    emit_av_kt(cur, kt - 2)
                a0 = pool.tile([128, N], BF16, tag="attn", bufs=26, name=f"at{h0}_{kt}")
                nc.scalar.activation(a0, ps0, Act.Exp, bias=mb_sb[:, kt : kt + 1])
                at0.append(a0)
                a1 = pool.tile([128, N], BF16, tag="attn", bufs=26, name=f"at{h1}_{kt}")
                nc.scalar.activation(a1, ps1, Act.Exp, bias=mb_sb[:, kt : kt + 1])
                at1.append(a1)
            for kt in (TT - 2, TT - 1):
                emit_av_kt(cur, kt)
            finish_pair(cur)

        # ---------------- proj ----------------
        for ot in range(CT):
            ps = psum.tile([128, N], F32, tag="psA", bufs=2, name=f"ps_p{ot}")
            for c in range(CT):
                for qc in range(2):
                    nc.tensor.matmul(
                        ps[:, qc * 512 : (qc + 1) * 512],
                        lhsT=pw[c][:, ot * 128 : (ot + 1) * 128],
                        rhs=aoT[c][:, qc * 512 : (qc + 1) * 512],
                        start=(c == 0),
                        stop=(c == CT - 1),
                    )
            osb = pool.tile([128, N], F32, tag="osb", bufs=2, name=f"o{ot}")
            nc.scalar.activation(osb, ps, Act.Identity, bias=pb_sb[:, ot : ot + 1])
            nc.sync.dma_start(out=out_d[ot * 128 : (ot + 1) * 128, :], in_=osb)

    nc.finalize()
    return nc


def kernel(x, mask, qkv_w, q_bias, v_bias, proj_w, proj_b, **_):
    x = np.asarray(x, np.float32)
    mask = np.asarray(mask)
    qkv_w = np.asarray(qkv_w, np.float32)
    q_bias = np.asarray(q_bias, np.float32)
    v_bias = np.asarray(v_bias, np.float32)
    proj_w = np.asarray(proj_w, np.float32)
    proj_b = np.asarray(proj_b, np.float32)

    wqkT = np.ascontiguousarray(qkv_w.T)  # [C, 3C]
    wqkT[:, :C] *= SCALE
    qkb = np.concatenate([q_bias * SCALE, np.zeros(C, np.float32)])
    pb_eff = (proj_b + proj_w @ v_bias).astype(np.float32)
    pwT = np.ascontiguousarray(proj_w.T)
    wqkT_bf = wqkT.astype(ml_dtypes.bfloat16)
    pwT_bf = pwT.astype(ml_dtypes.bfloat16)
    mb = np.where(mask, np.float32(MASK_NEG), np.float32(0.0)).astype(np.float32)

    if "nc" not in _CACHE:
        _CACHE["nc"] = _build_nc()
    nc = _CACHE["nc"]

    in_maps = []
    for b in range(B):
        in_maps.append(
            {
                "xT": np.ascontiguousarray(x[b].T).astype(ml_dtypes.bfloat16),
                "wqkT": wqkT_bf,
                "pwT": pwT_bf,
                "qkb": qkb,
                "mb": np.ascontiguousarray(mb[b]),
                "pb": pb_eff,
            }
        )

    _CACHE["last_in_maps"] = in_maps
    res = run_bass_kernel_spmd(nc, in_maps, list(range(B)))
    out = np.stack([res.results[b]["out"].T for b in range(B)], axis=0)
    return out.astype(np.float32)


if __name__ == "__main__":
    np.random.seed(0)
    x = np.random.randn(B, N, C).astype(np.float32)
    mask = np.random.randint(0, 2, (B, N)) > 0
    qkv_w = (np.random.randn(F3, C) * 0.02).astype(np.float32)
    q_bias = (np.random.randn(C) * 0.02).astype(np.float32)
    v_bias = (np.random.randn(C) * 0.02).astype(np.float32)
    proj_w = (np.random.randn(C, C) * 0.02).astype(np.float32)
    proj_b = (np.random.randn(C) * 0.02).astype(np.float32)
    out = kernel(x, mask, qkv_w, q_bias, v_bias, proj_w, proj_b)
    print(out.shape, out.dtype)


# revision 22
# speedup vs baseline: 1.1507x; 1.1507x over previous
"""Trainium2 Bass kernel for AttentionWithCAE.

Reference computation (B=8, N=1024, C=768, H=12, hd=64):
    qkv  = x @ qkv_w.T + concat(q_bias, 0, v_bias)
    q,k,v per head; attn = softmax(mask(q*scale @ k.T)); out = attn @ v
    final = out @ proj_w.T + proj_b

Sharding: pure data parallel — batch b on core b, weights replicated,
no collectives.

Device-side layout strategy (per core):
  - Host pre-transposes operands so the device kernel does zero transposes:
      xT [C, N], wqkT [C, 3C] (q-cols pre-scaled by SCALE), pwT [C, C],
      all cast to bf16 on the host (PSUM accumulation stays fp32).
  - qk projection emitted as qkT [1536, N] (feature-major): head h's qT/kT
    are rows h*64..h*64+64 — exactly the lhsT/rhs layout the scores matmul
    needs (contraction over head_dim).
  - v projection emitted token-major [N, 768] interleaved into v65 tiles
    [128, 12*65]: per head 64 v-columns plus a baked ones column, so one
    M=65 matmul per (head, k-tile, q-chunk) yields both attn@v and the
    softmax denominators (row 64 of PSUM).
  - scores computed transposed [k, q]: the key-dependent mask bias becomes a
    per-partition bias folded into the Exp activation (single ACT op;
    no max-subtraction needed: |scores| <= ~10 so exp can't overflow).
  - softmax denominators -> SBUF -> approx reciprocal -> partition-broadcast
    via a DRAM bounce (DMA broadcast needs a DRAM source).
  - attn output accumulates transposed [hd, t] which directly feeds the
    proj matmul; final output is [C, N] and the host transposes it back.
  - q_bias folds into the qkT eviction (per-partition bias); v_bias folds
    into an effective proj bias on the host (attn rows sum to 1).

Scheduling (the emission order shapes the per-engine execution order):
  - v-projection first, then per head-pair p: its two qkT tiles, then the
    pair's scores (row-packed: even head rows 0-63, odd head rows 64-127 ->
    concurrent K=64 matmuls), with the PREVIOUS pair's attn@v matmuls
    interleaved kt-by-kt. QKV work for pair p+1 fills PE gaps while ACT
    runs the exps of pair p, keeping the PE dense (no HAM re-throttle).
"""

import sys

sys.path.insert(0, "/opt/trn_rl_repo")

from contextlib import ExitStack

import numpy as np
import ml_dtypes

import concourse.bass as bass
import concourse.bacc as bacc
import concourse.mybir as mybir
from concourse import tile
from concourse.bass_utils import run_bass_kernel_spmd

B, N, C = 8, 1024, 768
H, HD = 12, 64
F3 = 3 * C  # 2304
SCALE = HD ** -0.5
F32 = mybir.dt.float32
BF16 = mybir.dt.bfloat16
Act = mybir.ActivationFunctionType

MASK_NEG = -30000.0

CT = C // 128  # 6 contraction tiles
TT = N // 128  # 8 token tiles
QKT = 2 * C // 128  # 12 qk feature tiles
NPAIR = H // 2  # 6 head pairs

_CACHE = {}


def _build_nc():
    nc = bacc.Bacc(None, target_bir_lowering=False)

    xT_d = nc.declare_dram_parameter("xT", [C, N], BF16, isOutput=False)
    wqk_d = nc.declare_dram_parameter("wqkT", [C, F3], BF16, isOutput=False)
    pw_d = nc.declare_dram_parameter("pwT", [C, C], BF16, isOutput=False)
    qkb_d = nc.declare_dram_parameter("qkb", [2 * C], F32, isOutput=False)
    mb_d = nc.declare_dram_parameter("mb", [N], F32, isOutput=False)
    pb_d = nc.declare_dram_parameter("pb", [C], F32, isOutput=False)
    out_d = nc.declare_dram_parameter("out", [C, N], F32, isOutput=True)

    r_d = nc.dram_tensor("r_scratch", [H, N], F32)

    with ExitStack() as ctx:
        tc = ctx.enter_context(tile.TileContext(nc))
        pool = ctx.enter_context(tc.tile_pool(name="main", bufs=1))
        psum = ctx.enter_context(tc.tile_pool(name="psum", bufs=1, space="PSUM"))

        F32R = mybir.dt.float32r
        ones64 = pool.tile([1, 64], F32R, tag="ones64", bufs=1)
        nc.vector.memset(ones64.bitcast(F32), 1.0)
        qkb_sb = pool.tile([128, QKT], F32)
        nc.sync.dma_start(out=qkb_sb, in_=qkb_d.rearrange("(i p) -> p i", p=128))
        mb_sb = pool.tile([128, TT], F32)
        nc.sync.dma_start(out=mb_sb, in_=mb_d.rearrange("(i p) -> p i", p=128))
        pb_sb = pool.tile([128, CT], F32)
        nc.sync.dma_start(out=pb_sb, in_=pb_d.rearrange("(i p) -> p i", p=128))

        wqk = []
        xTs = []
        for c in range(CT):
            w = pool.tile([128, F3], BF16, tag="wqk", bufs=CT, name=f"wqk{c}")
            for j in range(4):
                nc.sync.dma_start(
                    out=w[:, j * 576 : (j + 1) * 576],
                    in_=wqk_d[c * 128 : (c + 1) * 128, j * 576 : (j + 1) * 576],
                )
            wqk.append(w)
            xt = pool.tile([128, N], BF16, tag="xT", bufs=CT, name=f"xT{c}")
            for j in range(2):
                nc.sync.dma_start(
                    out=xt[:, j * 512 : (j + 1) * 512],
                    in_=xT_d[c * 128 : (c + 1) * 128, j * 512 : (j + 1) * 512],
                )
            xTs.append(xt)
        pw = []
        for c in range(CT):
            w = pool.tile([128, C], BF16, tag="pw", bufs=CT, name=f"pw{c}")
            nc.sync.dma_start(out=w, in_=pw_d[c * 128 : (c + 1) * 128, :])
            pw.append(w)

        qkT = [
            pool.tile([128, N], BF16, tag="qkT", bufs=QKT, name=f"qkT{i}")
            for i in range(QKT)
        ]
        v65 = [
            pool.tile([128, H * 65], BF16, tag="v65", bufs=TT, name=f"v65_{i}")
            for i in range(TT)
        ]
        aoT = [
            pool.tile([128, N], BF16, tag="aoT", bufs=CT, name=f"aoT{i}")
            for i in range(CT)
        ]

        # ---------------- v projection (needed by every head's AV) --------
        for ti in range(TT):
            psa = psum.tile([128, 512], F32, tag=f"psAV{ti % 2}0", bufs=1, name=f"ps_va{ti}")
            psb = psum.tile([128, 256], F32, tag=f"psAV{ti % 2}1", bufs=1, name=f"ps_vb{ti}")
            for c in range(CT):
                nc.tensor.matmul(
                    psa,
                    lhsT=xTs[c][:, ti * 128 : (ti + 1) * 128],
                    rhs=wqk[c][:, 1536:2048],
                    start=(c == 0),
                    stop=(c == CT - 1),
                )
                nc.tensor.matmul(
                    psb,
                    lhsT=xTs[c][:, ti * 128 : (ti + 1) * 128],
                    rhs=wqk[c][:, 2048:2304],
                    start=(c == 0),
                    stop=(c == CT - 1),
                )
            v3 = v65[ti].rearrange("p (h j) -> p h j", j=65)
            nc.scalar.activation(
                v3[:, 0:8, 0:64], psa.rearrange("p (h j) -> p h j", j=64), Act.Copy
            )
            nc.scalar.activation(
                v3[:, 8:12, 0:64], psb.rearrange("p (h j) -> p h j", j=64), Act.Copy
            )
            nc.vector.memset(v3[:, :, 64:65], 1.0)

        def emit_qk_tile(fi):
            ps = psum.tile([128, N], F32, tag="psA", bufs=2, name=f"ps_qk{fi}")
            for c in range(CT):
                for qc in range(2):
                    nc.tensor.matmul(
                        ps[:, qc * 512 : (qc + 1) * 512],
                        lhsT=wqk[c][:, fi * 128 : (fi + 1) * 128],
                        rhs=xTs[c][:, qc * 512 : (qc + 1) * 512],
                        start=(c == 0),
                        stop=(c == CT - 1),
                    )
            nc.vector.tensor_scalar_add(
                out=qkT[fi], in0=ps, scalar1=qkb_sb[:, fi : fi + 1]
            )

        def emit_av_kt(pr, kt):
            for hi, (h, atiles) in enumerate(
                [(pr["h0"], pr["at0"]), (pr["h1"], pr["at1"])]
            ):
                for qc in range(2):
                    nc.tensor.matmul(
                        pr["pav"][hi][qc][0:65, :],
                        lhsT=v65[kt][:, h * 65 : (h + 1) * 65],
                        rhs=atiles[kt][:, qc * 512 : (qc + 1) * 512],
                        start=(kt == 0),
                        stop=(kt == TT - 1),
                    )

        def finish_pair(pr):
            # Evict AV PSUM to SBUF right away (fast DVE copies release the
            # PSUM banks so the next pair's AV can start), then run the slow
            # normalization chain (recip -> DRAM-bounce broadcast -> mul)
            # entirely from SBUF.
            for hi, h in enumerate([pr["h0"], pr["h1"]]):
                qt, row = h // 2, (h % 2) * 64
                pav = pr["pav"][hi]
                un = [
                    pool.tile([64, 512], F32, tag=f"un{qc}", bufs=2, name=f"un{h}_{qc}")
                    for qc in range(2)
                ]
                srow = pool.tile([1, N], F32, tag="srow", bufs=2, name=f"s{h}")
                for qc in range(2):
                    nc.vector.tensor_copy(out=un[qc], in_=pav[qc][0:64, :])
                    nc.vector.tensor_copy(
                        out=srow[:, qc * 512 : (qc + 1) * 512], in_=pav[qc][64:65, :]
                    )
                r_row = pool.tile([1, N], F32R, tag="rrow", bufs=2, name=f"r{h}")
                r_f32 = pool.tile([1, N], F32, tag="rf", bufs=2, name=f"rf{h}")
                nc.vector.reciprocal_approx_fast(out=r_f32, in_=srow)
                nc.vector.tensor_copy(out=r_row, in_=r_f32)
                r2ps = psum.tile([64, N], F32, tag="psA", bufs=2, name=f"r2p{h}")
                for qc in range(2):
                    nc.tensor.matmul(
                        r2ps[:, qc * 512 : (qc + 1) * 512],
                        lhsT=ones64,
                        rhs=r_row[:, qc * 512 : (qc + 1) * 512],
                        start=True,
                        stop=True,
                    )
                for qc in range(2):
                    nc.vector.tensor_mul(
                        out=aoT[qt][row : row + 64, qc * 512 : (qc + 1) * 512],
                        in0=un[qc][0:64, :],
                        in1=r2ps[0:64, qc * 512 : (qc + 1) * 512],
                    )

        prev = None
        for p in range(NPAIR):
            emit_qk_tile(p)
            emit_qk_tile(CT + p)
            h0, h1 = 2 * p, 2 * p + 1
            k_tile, q_tile = qkT[CT + p], qkT[p]
            at0, at1 = [], []
            pav = []
            for hi in range(2):
                row = [
                    psum.tile(
                        [128, 512],
                        F32,
                        tag=f"psAV{hi}{qc}",
                        bufs=1,
                        name=f"pav{2 * p + hi}_{qc}",
                    )
                    for qc in range(2)
                ]
                pav.append(row)
            cur = {"h0": h0, "h1": h1, "at0": at0, "at1": at1, "pav": pav}
            for kt in range(TT):
                ps0 = psum.tile([128, N], F32, tag="psA", bufs=2, name=f"ps_s{h0}_{kt}")
                ps1 = psum.tile([128, N], F32, tag="psA", bufs=2, name=f"ps_s{h1}_{kt}")
                for qc in range(2):
                    # row-packed pair: even head rows 0-63, odd head rows 64-127
                    nc.tensor.matmul(
                        ps0[:, qc * 512 : (qc + 1) * 512],
                        lhsT=k_tile[0:64, kt * 128 : (kt + 1) * 128],
                        rhs=q_tile[0:64, qc * 512 : (qc + 1) * 512],
                        start=True,
                        stop=True,
                    )
                    nc.tensor.matmul(
                        ps1[:, qc * 512 : (qc + 1) * 512],
                        lhsT=k_tile[64:128, kt * 128 : (kt + 1) * 128],
                        rhs=q_tile[64:128, qc * 512 : (qc + 1) * 512],
                        start=True,
                        stop=True,
                    )
                if prev is not None:
                    emit_av_kt(prev, kt)
                a0 = pool.tile([128, N], BF16, tag="attn", bufs=26, name=f"at{h0}_{kt}")
                nc.scalar.activation(a0, ps0, Act.Exp, bias=mb_sb[:, kt : kt + 1])
                at0.append(a0)
                a1 = pool.tile([128, N], BF16, tag="attn", bufs=26, name=f"at{h1}_{kt}")
                nc.scalar.activation(a1, ps1, Act.Exp, bias=mb_sb[:, kt : kt + 1])
                at1.append(a1)
            if prev is not None:
                finish_pair(prev)
            prev = cur

        # drain: AV + normalization for the last pair
        for kt in range(TT):
            emit_av_kt(prev, kt)
        finish_pair(prev)

        # ---------------- proj ----------------
        for ot in range(CT):
            ps = psum.tile([128, N], F32, tag="psA", bufs=2, name=f"ps_p{ot}")
            for c in range(CT):
                for qc in range(2):
                    nc.tensor.matmul(
                        ps[:, qc * 512 : (qc + 1) * 512],
                        lhsT=pw[c][:, ot * 128 : (ot + 1) * 128],
                        rhs=aoT[c][:, qc * 512 : (qc + 1) * 512],
                        start=(c == 0),
                        stop=(c == CT - 1),
                    )
            osb = pool.tile([128, N], F32, tag="osb", bufs=2, name=f"o{ot}")
            nc.scalar.activation(osb, ps, Act.Identity, bias=pb_sb[:, ot : ot + 1])
            nc.sync.dma_start(out=out_d[ot * 128 : (ot + 1) * 128, :], in_=osb)

    nc.finalize()
    return nc


def kernel(x, mask, qkv_w, q_bias, v_bias, proj_w, proj_b, **_):
    x = np.asarray(x, np.float32)
    mask = np.asarray(mask)
    qkv_w = np.asarray(qkv_w, np.float32)
    q_bias = np.asarray(q_bias, np.float32)
    v_bias = np.asarray(v_bias, np.float32)
    proj_w = np.asarray(proj_w, np.float32)
    proj_b = np.asarray(proj_b, np.float32)

    wqkT = np.ascontiguousarray(qkv_w.T)  # [C, 3C]
    wqkT[:, :C] *= SCALE
    qkb = np.concatenate([q_bias * SCALE, np.zeros(C, np.float32)])
    pb_eff = (proj_b + proj_w @ v_bias).astype(np.float32)
    pwT = np.ascontiguousarray(proj_w.T)
    wqkT_bf = wqkT.astype(ml_dtypes.bfloat16)
    pwT_bf = pwT.astype(ml_dtypes.bfloat16)
    mb = np.where(mask, np.float32(MASK_NEG), np.float32(0.0)).astype(np.float32)

    if "nc" not in _CACHE:
        _CACHE["nc"] = _build_nc()
    nc = _CACHE["nc"]

    in_maps = []
    for b in range(B):
        in_maps.append(
            {
                "xT": np.ascontiguousarray(x[b].T).astype(ml_dtypes.bfloat16),
                "wqkT": wqkT_bf,
                "pwT": pwT_bf,
                "qkb": qkb,
                "mb": np.ascontiguousarray(mb[b]),
                "pb": pb_eff,
            }
        )

    _CACHE["last_in_maps"] = in_maps
    res = run_bass_kernel_spmd(nc, in_maps, list(range(B)))
    out = np.stack([res.results[b]["out"].T for b in range(B)], axis=0)
    return out.astype(np.float32)


if __name__ == "__main__":
    np.random.seed(0)
    x = np.random.randn(B, N, C).astype(np.float32)
    mask = np.random.randint(0, 2, (B, N)) > 0
    qkv_w = (np.random.randn(F3, C) * 0.02).astype(np.float32)
    q_bias = (np.random.randn(C) * 0.02).astype(np.float32)
    v_bias = (np.random.randn(C) * 0.02).astype(np.float32)
    proj_w = (np.random.randn(C, C) * 0.02).astype(np.float32)
    proj_b = (np.random.randn(C) * 0.02).astype(np.float32)
    out = kernel(x, mask, qkv_w, q_bias, v_bias, proj_w, proj_b)
    print(out.shape, out.dtype)


# revision 23
# speedup vs baseline: 1.4596x; 1.2684x over previous
"""Trainium2 Bass kernel for AttentionWithCAE.

Reference computation (B=8, N=1024, C=768, H=12, hd=64):
    qkv  = x @ qkv_w.T + concat(q_bias, 0, v_bias)
    q,k,v per head; attn = softmax(mask(q*scale @ k.T)); out = attn @ v
    final = out @ proj_w.T + proj_b

Sharding: pure data parallel — batch b on core b, weights replicated,
no collectives.

Device-side layout strategy (per core):
  - Host pre-transposes operands so the device kernel does zero transposes:
      xT [C, N], wqkT [C, 3C] (q-cols pre-scaled by SCALE), pwT [C, C],
      all cast to bf16 on the host (PSUM accumulation stays fp32).
  - qk projection emitted as qkT [1536, N] (feature-major): head h's qT/kT
    are rows h*64..h*64+64 — exactly the lhsT/rhs layout the scores matmul
    needs (contraction over head_dim).
  - v projection emitted token-major [N, 768] interleaved into v65 tiles
    [128, 12*65]: per head 64 v-columns plus a baked ones column, so one
    M=65 matmul per (head, k-tile, q-chunk) yields both attn@v and the
    softmax denominators (row 64 of PSUM).
  - scores computed transposed [k, q]: the key-dependent mask bias becomes a
    per-partition bias folded into the Exp activation (single ACT op;
    no max-subtraction needed: |scores| <= ~10 so exp can't overflow).
  - softmax denominators -> SBUF -> approx reciprocal -> partition-broadcast
    via a DRAM bounce (DMA broadcast needs a DRAM source).
  - attn output accumulates transposed [hd, t] which directly feeds the
    proj matmul; final output is [C, N] and the host transposes it back.
  - q_bias folds into the qkT eviction (per-partition bias); v_bias folds
    into an effective proj bias on the host (attn rows sum to 1).

Scheduling (the emission order shapes the per-engine execution order):
  - v-projection first, then per head-pair p: its two qkT tiles, then the
    pair's scores (row-packed: even head rows 0-63, odd head rows 64-127 ->
    concurrent K=64 matmuls), with the PREVIOUS pair's attn@v matmuls
    interleaved kt-by-kt. QKV work for pair p+1 fills PE gaps while ACT
    runs the exps of pair p, keeping the PE dense (no HAM re-throttle).
"""

import sys

sys.path.insert(0, "/opt/trn_rl_repo")

from contextlib import ExitStack

import numpy as np
import ml_dtypes

import concourse.bass as bass
import concourse.bacc as bacc
import concourse.mybir as mybir
from concourse import tile
from concourse.bass_utils import run_bass_kernel_spmd

B, N, C = 8, 1024, 768
H, HD = 12, 64
F3 = 3 * C  # 2304
SCALE = HD ** -0.5
F32 = mybir.dt.float32
BF16 = mybir.dt.bfloat16
Act = mybir.ActivationFunctionType

MASK_NEG = -30000.0

CT = C // 128  # 6 contraction tiles
TT = N // 128  # 8 token tiles
QKT = 2 * C // 128  # 12 qk feature tiles
NPAIR = H // 2  # 6 head pairs

_CACHE = {}


def _build_nc():
    nc = bacc.Bacc(None, target_bir_lowering=False)

    xT_d = nc.declare_dram_parameter("xT", [C, N], BF16, isOutput=False)
    wqk_d = nc.declare_dram_parameter("wqkT", [C, F3], BF16, isOutput=False)
    pw_d = nc.declare_dram_parameter("pwT", [C, C], BF16, isOutput=False)
    qkb_d = nc.declare_dram_parameter("qkb", [2 * C], F32, isOutput=False)
    mb_d = nc.declare_dram_parameter("mb", [N], F32, isOutput=False)
    pb_d = nc.declare_dram_parameter("pb", [C], F32, isOutput=False)
    out_d = nc.declare_dram_parameter("out", [C, N], F32, isOutput=True)

    r_d = nc.dram_tensor("r_scratch", [H, N], F32)

    with ExitStack() as ctx:
        tc = ctx.enter_context(tile.TileContext(nc))
        pool = ctx.enter_context(tc.tile_pool(name="main", bufs=1))
        psum = ctx.enter_context(tc.tile_pool(name="psum", bufs=1, space="PSUM"))

        qkb_sb = pool.tile([128, QKT], F32)
        nc.sync.dma_start(out=qkb_sb, in_=qkb_d.rearrange("(i p) -> p i", p=128))
        mb_sb = pool.tile([128, TT], F32)
        nc.sync.dma_start(out=mb_sb, in_=mb_d.rearrange("(i p) -> p i", p=128))
        pb_sb = pool.tile([128, CT], F32)
        nc.sync.dma_start(out=pb_sb, in_=pb_d.rearrange("(i p) -> p i", p=128))

        # inputs: partition-split DMAs so chunks land on parallel queues
        # while DRAM rows stay whole (no descriptor fragmentation)
        wqk = []
        xTs = []
        for c in range(CT):
            w = pool.tile([128, F3], BF16, tag="wqk", bufs=CT, name=f"wqk{c}")
            for j in range(4):
                nc.sync.dma_start(
                    out=w[j * 32 : (j + 1) * 32, :],
                    in_=wqk_d[c * 128 + j * 32 : c * 128 + (j + 1) * 32, :],
                )
            wqk.append(w)
            xt = pool.tile([128, N], BF16, tag="xT", bufs=CT, name=f"xT{c}")
            for j in range(2):
                nc.sync.dma_start(
                    out=xt[j * 64 : (j + 1) * 64, :],
                    in_=xT_d[c * 128 + j * 64 : c * 128 + (j + 1) * 64, :],
                )
            xTs.append(xt)
        pw = []
        for c in range(CT):
            w = pool.tile([128, C], BF16, tag="pw", bufs=CT, name=f"pw{c}")
            nc.sync.dma_start(out=w, in_=pw_d[c * 128 : (c + 1) * 128, :])
            pw.append(w)

        qkT = [
            pool.tile([128, N], BF16, tag="qkT", bufs=QKT, name=f"qkT{i}")
            for i in range(QKT)
        ]
        v65 = [
            pool.tile([128, H * 65], BF16, tag="v65", bufs=TT, name=f"v65_{i}")
            for i in range(TT)
        ]
        aoT = [
            pool.tile([128, N], BF16, tag="aoT", bufs=CT, name=f"aoT{i}")
            for i in range(CT)
        ]

        def emit_v_tile(ti):
            psa = psum.tile(
                [128, 512], F32, tag=f"psAV{ti % 2}0", bufs=1, name=f"ps_va{ti}"
            )
            psb = psum.tile(
                [128, 256], F32, tag=f"psAV{ti % 2}1", bufs=1, name=f"ps_vb{ti}"
            )
            for c in range(CT):
                nc.tensor.matmul(
                    psa,
                    lhsT=xTs[c][:, ti * 128 : (ti + 1) * 128],
                    rhs=wqk[c][:, 1536:2048],
                    start=(c == 0),
                    stop=(c == CT - 1),
                )
                nc.tensor.matmul(
                    psb,
                    lhsT=xTs[c][:, ti * 128 : (ti + 1) * 128],
                    rhs=wqk[c][:, 2048:2304],
                    start=(c == 0),
                    stop=(c == CT - 1),
                )
            v3 = v65[ti].rearrange("p (h j) -> p h j", j=65)
            nc.scalar.activation(
                v3[:, 0:8, 0:64], psa.rearrange("p (h j) -> p h j", j=64), Act.Copy
            )
            nc.scalar.activation(
                v3[:, 8:12, 0:64], psb.rearrange("p (h j) -> p h j", j=64), Act.Copy
            )
            nc.vector.memset(v3[:, :, 64:65], 1.0)

        def emit_qk_tile(fi):
            ps = psum.tile([128, N], F32, tag="psA", bufs=2, name=f"ps_qk{fi}")
            for c in range(CT):
                for qc in range(2):
                    nc.tensor.matmul(
                        ps[:, qc * 512 : (qc + 1) * 512],
                        lhsT=wqk[c][:, fi * 128 : (fi + 1) * 128],
                        rhs=xTs[c][:, qc * 512 : (qc + 1) * 512],
                        start=(c == 0),
                        stop=(c == CT - 1),
                    )
            nc.vector.tensor_scalar_add(
                out=qkT[fi], in0=ps, scalar1=qkb_sb[:, fi : fi + 1]
            )

        def emit_av_kt(pr, kt):
            for hi, (h, atiles) in enumerate(
                [(pr["h0"], pr["at0"]), (pr["h1"], pr["at1"])]
            ):
                for qc in range(2):
                    nc.tensor.matmul(
                        pr["pav"][hi][qc][0:65, :],
                        lhsT=v65[kt][:, h * 65 : (h + 1) * 65],
                        rhs=atiles[kt][:, qc * 512 : (qc + 1) * 512],
                        start=(kt == 0),
                        stop=(kt == TT - 1),
                    )

        def finish_pair(pr):
            # Evict AV PSUM to SBUF right away (fast DVE copies release the
            # PSUM banks so the next pair's AV can start), then run the slow
            # normalization chain (recip -> DRAM-bounce broadcast -> mul)
            # entirely from SBUF, off the PE stream.
            for hi, h in enumerate([pr["h0"], pr["h1"]]):
                qt, row = h // 2, (h % 2) * 64
                pav = pr["pav"][hi]
                un = [
                    pool.tile([64, 512], F32, tag=f"un{qc}", bufs=2, name=f"un{h}_{qc}")
                    for qc in range(2)
                ]
                srow = pool.tile([1, N], F32, tag="srow", bufs=2, name=f"s{h}")
                for qc in range(2):
                    nc.vector.tensor_copy(out=un[qc], in_=pav[qc][0:64, :])
                    nc.vector.tensor_copy(
                        out=srow[:, qc * 512 : (qc + 1) * 512], in_=pav[qc][64:65, :]
                    )
                r_row = pool.tile([1, N], F32, tag="rrow", bufs=2, name=f"r{h}")
                nc.vector.reciprocal_approx_fast(out=r_row, in_=srow)
                nc.sync.dma_start(out=r_d[h : h + 1, :], in_=r_row)
                r2 = pool.tile([64, N], F32, tag="r2", bufs=3, name=f"r2_{h}")
                nc.sync.dma_start(out=r2, in_=r_d[h : h + 1, :].to_broadcast([64, N]))
                for qc in range(2):
                    nc.vector.tensor_mul(
                        out=aoT[qt][row : row + 64, qc * 512 : (qc + 1) * 512],
                        in0=un[qc][0:64, :],
                        in1=r2[:, qc * 512 : (qc + 1) * 512],
                    )

        prev = None
        for p in range(NPAIR):
            emit_qk_tile(p)
            emit_qk_tile(CT + p)
            h0, h1 = 2 * p, 2 * p + 1
            k_tile, q_tile = qkT[CT + p], qkT[p]
            at0, at1 = [], []
            pav = []
            for hi in range(2):
                row = [
                    psum.tile(
                        [128, 512],
                        F32,
                        tag=f"psAV{hi}{qc}",
                        bufs=1,
                        name=f"pav{2 * p + hi}_{qc}",
                    )
                    for qc in range(2)
                ]
                pav.append(row)
            cur = {"h0": h0, "h1": h1, "at0": at0, "at1": at1, "pav": pav}
            for kt in range(TT):
                ps0 = psum.tile([128, N], F32, tag="psA", bufs=2, name=f"ps_s{h0}_{kt}")
                ps1 = psum.tile([128, N], F32, tag="psA", bufs=2, name=f"ps_s{h1}_{kt}")
                for qc in range(2):
                    # row-packed pair: even head rows 0-63, odd head rows 64-127
                    nc.tensor.matmul(
                        ps0[:, qc * 512 : (qc + 1) * 512],
                        lhsT=k_tile[0:64, kt * 128 : (kt + 1) * 128],
                        rhs=q_tile[0:64, qc * 512 : (qc + 1) * 512],
                        start=True,
                        stop=True,
                    )
                    nc.tensor.matmul(
                        ps1[:, qc * 512 : (qc + 1) * 512],
                        lhsT=k_tile[64:128, kt * 128 : (kt + 1) * 128],
                        rhs=q_tile[64:128, qc * 512 : (qc + 1) * 512],
                        start=True,
                        stop=True,
                    )
                if prev is not None:
                    emit_av_kt(prev, kt)
                if p == 0:
                    # pair 0 has no previous pair's AV to fill PE gaps --
                    # interleave the v-projection (needed from pair 1 on)
                    emit_v_tile(kt)
                a0 = pool.tile([128, N], BF16, tag="attn", bufs=26, name=f"at{h0}_{kt}")
                nc.scalar.activation(a0, ps0, Act.Exp, bias=mb_sb[:, kt : kt + 1])
                at0.append(a0)
                a1 = pool.tile([128, N], BF16, tag="attn", bufs=26, name=f"at{h1}_{kt}")
                nc.scalar.activation(a1, ps1, Act.Exp, bias=mb_sb[:, kt : kt + 1])
                at1.append(a1)
            if prev is not None:
                finish_pair(prev)
            prev = cur

        # drain: AV + normalization for the last pair
        for kt in range(TT):
            emit_av_kt(prev, kt)
        finish_pair(prev)

        # ---------------- proj ----------------
        for ot in range(CT):
            ps = psum.tile([128, N], F32, tag="psA", bufs=2, name=f"ps_p{ot}")
            for c in range(CT):
                for qc in range(2):
                    nc.tensor.matmul(
                        ps[:, qc * 512 : (qc + 1) * 512],
                        lhsT=pw[c][:, ot * 128 : (ot + 1) * 128],
                        rhs=aoT[c][:, qc * 512 : (qc + 1) * 512],
                        start=(c == 0),
                        stop=(c == CT - 1),
                    )
            osb = pool.tile([128, N], F32, tag="osb", bufs=2, name=f"o{ot}")
            nc.scalar.activation(osb, ps, Act.Identity, bias=pb_sb[:, ot : ot + 1])
            nc.sync.dma_start(out=out_d[ot * 128 : (ot + 1) * 128, :], in_=osb)

    nc.finalize()
    return nc


def kernel(x, mask, qkv_w, q_bias, v_bias, proj_w, proj_b, **_):
    x = np.asarray(x, np.float32)
    mask = np.asarray(mask)
    qkv_w = np.asarray(qkv_w, np.float32)
    q_bias = np.asarray(q_bias, np.float32)
    v_bias = np.asarray(v_bias, np.float32)
    proj_w = np.asarray(proj_w, np.float32)
    proj_b = np.asarray(proj_b, np.float32)

    wqkT = np.ascontiguousarray(qkv_w.T)  # [C, 3C]
    wqkT[:, :C] *= SCALE
    qkb = np.concatenate([q_bias * SCALE, np.zeros(C, np.float32)])
    pb_eff = (proj_b + proj_w @ v_bias).astype(np.float32)
    pwT = np.ascontiguousarray(proj_w.T)
    wqkT_bf = wqkT.astype(ml_dtypes.bfloat16)
    pwT_bf = pwT.astype(ml_dtypes.bfloat16)
    mb = np.where(mask, np.float32(MASK_NEG), np.float32(0.0)).astype(np.float32)

    if "nc" not in _CACHE:
        _CACHE["nc"] = _build_nc()
    nc = _CACHE["nc"]

    in_maps = []
    for b in range(B):
        in_maps.append(
            {
                "xT": np.ascontiguousarray(x[b].T).astype(ml_dtypes.bfloat16),
                "wqkT": wqkT_bf,
                "pwT": pwT_bf,
                "qkb": qkb,
                "mb": np.ascontiguousarray(mb[b]),
                "pb": pb_eff,
            }
        )

    _CACHE["last_in_maps"] = in_maps
    res = run_bass_kernel_spmd(nc, in_maps, list(range(B)))
    out = np.stack([res.results[b]["out"].T for b in range(B)], axis=0)
    return out.astype(np.float32)


if __name__ == "__main__":
    np.random.seed(0)
    x = np.random.randn(B, N, C).astype(np.float32)
    mask = np.random.randint(0, 2, (B, N)) > 0
    qkv_w = (np.random.randn(F3, C) * 0.02).astype(np.float32)
    q_bias = (np.random.randn(C) * 0.02).astype(np.float32)
    v_bias = (np.random.randn(C) * 0.02).astype(np.float32)
    proj_w = (np.random.randn(C, C) * 0.02).astype(np.float32)
    proj_b = (np.random.randn(C) * 0.02).astype(np.float32)
    out = kernel(x, mask, qkv_w, q_bias, v_bias, proj_w, proj_b)
    print(out.shape, out.dtype)


# revision 24
# speedup vs baseline: 1.6365x; 1.1212x over previous
"""Trainium2 Bass kernel for AttentionWithCAE.

Reference computation (B=8, N=1024, C=768, H=12, hd=64):
    qkv  = x @ qkv_w.T + concat(q_bias, 0, v_bias)
    q,k,v per head; attn = softmax(mask(q*scale @ k.T)); out = attn @ v
    final = out @ proj_w.T + proj_b

Sharding: pure data parallel — batch b on core b, weights replicated,
no collectives.

Device-side layout strategy (per core):
  - Host pre-transposes operands so the device kernel does zero transposes:
      xT [C, N], wqkT [C, 3C] (q-cols pre-scaled by SCALE), pwT [C, C],
      all cast to bf16 on the host (PSUM accumulation stays fp32).
  - qk projection emitted as qkT [1536, N] (feature-major): head h's qT/kT
    are rows h*64..h*64+64 — exactly the lhsT/rhs layout the scores matmul
    needs (contraction over head_dim).
  - v projection emitted token-major [N, 768] interleaved into v65 tiles
    [128, 12*65]: per head 64 v-columns plus a baked ones column, so one
    M=65 matmul per (head, k-tile, q-chunk) yields both attn@v and the
    softmax denominators (row 64 of PSUM).
  - scores computed transposed [k, q]: the key-dependent mask bias becomes a
    per-partition bias folded into the Exp activation (single ACT op;
    no max-subtraction needed: |scores| <= ~10 so exp can't overflow).
  - softmax denominators -> SBUF -> approx reciprocal -> partition-broadcast
    via a DRAM bounce (DMA broadcast needs a DRAM source).
  - attn output accumulates transposed [hd, t] which directly feeds the
    proj matmul; final output is [C, N] and the host transposes it back.
  - q_bias folds into the qkT eviction (per-partition bias); v_bias folds
    into an effective proj bias on the host (attn rows sum to 1).

Scheduling (the emission order shapes the per-engine execution order):
  - v-projection first, then per head-pair p: its two qkT tiles, then the
    pair's scores (row-packed: even head rows 0-63, odd head rows 64-127 ->
    concurrent K=64 matmuls), with the PREVIOUS pair's attn@v matmuls
    interleaved kt-by-kt. QKV work for pair p+1 fills PE gaps while ACT
    runs the exps of pair p, keeping the PE dense (no HAM re-throttle).
"""

import sys

sys.path.insert(0, "/opt/trn_rl_repo")

from contextlib import ExitStack

import numpy as np
import ml_dtypes

import concourse.bass as bass
import concourse.bacc as bacc
import concourse.mybir as mybir
from concourse import tile
from concourse.bass_utils import run_bass_kernel_spmd

B, N, C = 8, 1024, 768
H, HD = 12, 64
F3 = 3 * C  # 2304
SCALE = HD ** -0.5
F32 = mybir.dt.float32
BF16 = mybir.dt.bfloat16
Act = mybir.ActivationFunctionType

MASK_NEG = -30000.0

CT = C // 128  # 6 contraction tiles
TT = N // 128  # 8 token tiles
QKT = 2 * C // 128  # 12 qk feature tiles
NPAIR = H // 2  # 6 head pairs

_CACHE = {}


def _build_nc():
    nc = bacc.Bacc(None, target_bir_lowering=False)

    xT_d = nc.declare_dram_parameter("xT", [C, N], BF16, isOutput=False)
    wqk_d = nc.declare_dram_parameter("wqkT", [C, F3], BF16, isOutput=False)
    pw_d = nc.declare_dram_parameter("pwT", [C, C], BF16, isOutput=False)
    qkb_d = nc.declare_dram_parameter("qkb", [2 * C], F32, isOutput=False)
    mb_d = nc.declare_dram_parameter("mb", [N], F32, isOutput=False)
    pb_d = nc.declare_dram_parameter("pb", [C], F32, isOutput=False)
    out_d = nc.declare_dram_parameter("out", [C, N], F32, isOutput=True)

    r_d = nc.dram_tensor("r_scratch", [H, N], F32)

    with ExitStack() as ctx:
        tc = ctx.enter_context(tile.TileContext(nc))
        pool = ctx.enter_context(tc.tile_pool(name="main", bufs=1))
        psum = ctx.enter_context(tc.tile_pool(name="psum", bufs=1, space="PSUM"))

        qkb_sb = pool.tile([128, QKT], F32)
        nc.sync.dma_start(out=qkb_sb, in_=qkb_d.rearrange("(i p) -> p i", p=128))
        mb_sb = pool.tile([128, TT], F32)
        nc.sync.dma_start(out=mb_sb, in_=mb_d.rearrange("(i p) -> p i", p=128))
        pb_sb = pool.tile([128, CT], F32)
        nc.sync.dma_start(out=pb_sb, in_=pb_d.rearrange("(i p) -> p i", p=128))

        # inputs: partition-split DMAs so chunks land on parallel queues
        # while DRAM rows stay whole (no descriptor fragmentation)
        wqk = []
        xTs = []
        for c in range(CT):
            w = pool.tile([128, F3], BF16, tag="wqk", bufs=CT, name=f"wqk{c}")
            for j in range(2):
                nc.sync.dma_start(
                    out=w[:, j * 1152 : (j + 1) * 1152],
                    in_=wqk_d[c * 128 : (c + 1) * 128, j * 1152 : (j + 1) * 1152],
                )
            wqk.append(w)
            xt = pool.tile([128, N], BF16, tag="xT", bufs=CT, name=f"xT{c}")
            nc.sync.dma_start(out=xt, in_=xT_d[c * 128 : (c + 1) * 128, :])
            xTs.append(xt)
        pw = []
        for c in range(CT):
            w = pool.tile([128, C], BF16, tag="pw", bufs=CT, name=f"pw{c}")
            nc.sync.dma_start(out=w, in_=pw_d[c * 128 : (c + 1) * 128, :])
            pw.append(w)

        qkT = [
            pool.tile([128, N], BF16, tag="qkT", bufs=QKT, name=f"qkT{i}")
            for i in range(QKT)
        ]
        v65 = [
            pool.tile([128, H * 65], BF16, tag="v65", bufs=TT, name=f"v65_{i}")
            for i in range(TT)
        ]
        aoT = [
            pool.tile([128, N], BF16, tag="aoT", bufs=CT, name=f"aoT{i}")
            for i in range(CT)
        ]

        def emit_v_tile(ti):
            psa = psum.tile(
                [128, 512], F32, tag=f"psAV{ti % 2}0", bufs=1, name=f"ps_va{ti}"
            )
            psb = psum.tile(
                [128, 256], F32, tag=f"psAV{ti % 2}1", bufs=1, name=f"ps_vb{ti}"
            )
            for c in range(CT):
                nc.tensor.matmul(
                    psa,
                    lhsT=xTs[c][:, ti * 128 : (ti + 1) * 128],
                    rhs=wqk[c][:, 1536:2048],
                    start=(c == 0),
                    stop=(c == CT - 1),
                )
                nc.tensor.matmul(
                    psb,
                    lhsT=xTs[c][:, ti * 128 : (ti + 1) * 128],
                    rhs=wqk[c][:, 2048:2304],
                    start=(c == 0),
                    stop=(c == CT - 1),
                )
            v3 = v65[ti].rearrange("p (h j) -> p h j", j=65)
            nc.scalar.activation(
                v3[:, 0:8, 0:64], psa.rearrange("p (h j) -> p h j", j=64), Act.Copy
            )
            nc.scalar.activation(
                v3[:, 8:12, 0:64], psb.rearrange("p (h j) -> p h j", j=64), Act.Copy
            )
            nc.vector.memset(v3[:, :, 64:65], 1.0)

        def emit_qk_tile(fi):
            ps = psum.tile([128, N], F32, tag="psA", bufs=2, name=f"ps_qk{fi}")
            for c in range(CT):
                for qc in range(2):
                    nc.tensor.matmul(
                        ps[:, qc * 512 : (qc + 1) * 512],
                        lhsT=wqk[c][:, fi * 128 : (fi + 1) * 128],
                        rhs=xTs[c][:, qc * 512 : (qc + 1) * 512],
                        start=(c == 0),
                        stop=(c == CT - 1),
                    )
            nc.vector.tensor_scalar_add(
                out=qkT[fi], in0=ps, scalar1=qkb_sb[:, fi : fi + 1]
            )

        def emit_av_kt(pr, kt):
            for hi, (h, atiles) in enumerate(
                [(pr["h0"], pr["at0"]), (pr["h1"], pr["at1"])]
            ):
                for qc in range(2):
                    nc.tensor.matmul(
                        pr["pav"][hi][qc][0:65, :],
                        lhsT=v65[kt][:, h * 65 : (h + 1) * 65],
                        rhs=atiles[kt][:, qc * 512 : (qc + 1) * 512],
                        start=(kt == 0),
                        stop=(kt == TT - 1),
                    )

        def finish_pair(pr):
            # Evict AV PSUM to SBUF right away (fast DVE copies release the
            # PSUM banks so the next pair's AV can start), then run the slow
            # normalization chain (recip -> DRAM-bounce broadcast -> mul)
            # entirely from SBUF, off the PE stream.
            for hi, h in enumerate([pr["h0"], pr["h1"]]):
                qt, row = h // 2, (h % 2) * 64
                pav = pr["pav"][hi]
                un = [
                    pool.tile([64, 512], F32, tag=f"un{qc}", bufs=2, name=f"un{h}_{qc}")
                    for qc in range(2)
                ]
                srow = pool.tile([1, N], F32, tag="srow", bufs=2, name=f"s{h}")
                for qc in range(2):
                    nc.vector.tensor_copy(out=un[qc], in_=pav[qc][0:64, :])
                    nc.vector.tensor_copy(
                        out=srow[:, qc * 512 : (qc + 1) * 512], in_=pav[qc][64:65, :]
                    )
                r_row = pool.tile([1, N], F32, tag="rrow", bufs=2, name=f"r{h}")
                nc.vector.reciprocal_approx_fast(out=r_row, in_=srow)
                nc.sync.dma_start(out=r_d[h : h + 1, :], in_=r_row)
                r2 = pool.tile([64, N], F32, tag="r2", bufs=3, name=f"r2_{h}")
                nc.sync.dma_start(out=r2, in_=r_d[h : h + 1, :].to_broadcast([64, N]))
                for qc in range(2):
                    nc.vector.tensor_mul(
                        out=aoT[qt][row : row + 64, qc * 512 : (qc + 1) * 512],
                        in0=un[qc][0:64, :],
                        in1=r2[:, qc * 512 : (qc + 1) * 512],
                    )

        prev = None
        for p in range(NPAIR):
            emit_qk_tile(p)
            emit_qk_tile(CT + p)
            h0, h1 = 2 * p, 2 * p + 1
            k_tile, q_tile = qkT[CT + p], qkT[p]
            at0, at1 = [], []
            pav = []
            for hi in range(2):
                row = [
                    psum.tile(
                        [128, 512],
                        F32,
                        tag=f"psAV{hi}{qc}",
                        bufs=1,
                        name=f"pav{2 * p + hi}_{qc}",
                    )
                    for qc in range(2)
                ]
                pav.append(row)
            cur = {"h0": h0, "h1": h1, "at0": at0, "at1": at1, "pav": pav}
            for kt in range(TT):
                ps0 = psum.tile([128, N], F32, tag="psA", bufs=2, name=f"ps_s{h0}_{kt}")
                ps1 = psum.tile([128, N], F32, tag="psA", bufs=2, name=f"ps_s{h1}_{kt}")
                for qc in range(2):
                    # row-packed pair: even head rows 0-63, odd head rows 64-127
                    nc.tensor.matmul(
                        ps0[:, qc * 512 : (qc + 1) * 512],
                        lhsT=k_tile[0:64, kt * 128 : (kt + 1) * 128],
                        rhs=q_tile[0:64, qc * 512 : (qc + 1) * 512],
                        start=True,
                        stop=True,
                    )
                    nc.tensor.matmul(
                        ps1[:, qc * 512 : (qc + 1) * 512],
                        lhsT=k_tile[64:128, kt * 128 : (kt + 1) * 128],
                        rhs=q_tile[64:128, qc * 512 : (qc + 1) * 512],
                        start=True,
                        stop=True,
                    )
                if prev is not None:
                    emit_av_kt(prev, kt)
                if p == 0:
                    # pair 0 has no previous pair's AV to fill PE gaps --
                    # interleave the v-projection (needed from pair 1 on)
                    emit_v_tile(kt)
                a0 = pool.tile([128, N], BF16, tag="attn", bufs=26, name=f"at{h0}_{kt}")
                nc.scalar.activation(a0, ps0, Act.Exp, bias=mb_sb[:, kt : kt + 1])
                at0.append(a0)
                a1 = pool.tile([128, N], BF16, tag="attn", bufs=26, name=f"at{h1}_{kt}")
                nc.scalar.activation(a1, ps1, Act.Exp, bias=mb_sb[:, kt : kt + 1])
                at1.append(a1)
            if prev is not None:
                finish_pair(prev)
            prev = cur

        # drain: AV + normalization for the last pair
        for kt in range(TT):
            emit_av_kt(prev, kt)
        finish_pair(prev)

        # warm-keeper: dependency-free matmuls keep the PE busy (and the HAM
        # clock-gate open) while the last normalization chain runs, so proj
        # starts at full clock. Results are never read.
        for wi in range(24):
            wps = psum.tile([128, 512], F32, tag="psAV00", bufs=1, name=f"warm{wi}")
            nc.tensor.matmul(
                wps,
                lhsT=wqk[wi % CT][:, 0:128],
                rhs=xTs[wi % CT][:, 0:512],
                start=True,
                stop=True,
            )

        # ---------------- proj ----------------
        for ot in range(CT):
            ps = psum.tile([128, N], F32, tag="psA", bufs=2, name=f"ps_p{ot}")
            for c in range(CT):
                for qc in range(2):
                    nc.tensor.matmul(
                        ps[:, qc * 512 : (qc + 1) * 512],
                        lhsT=pw[c][:, ot * 128 : (ot + 1) * 128],
                        rhs=aoT[c][:, qc * 512 : (qc + 1) * 512],
                        start=(c == 0),
                        stop=(c == CT - 1),
                    )
            osb = pool.tile([128, N], F32, tag="osb", bufs=2, name=f"o{ot}")
            nc.scalar.activation(osb, ps, Act.Identity, bias=pb_sb[:, ot : ot + 1])
            nc.sync.dma_start(out=out_d[ot * 128 : (ot + 1) * 128, :], in_=osb)

    nc.finalize()
    return nc


def kernel(x, mask, qkv_w, q_bias, v_bias, proj_w, proj_b, **_):
    x = np.asarray(x, np.float32)
    mask = np.asarray(mask)
    qkv_w = np.asarray(qkv_w, np.float32)
    q_bias = np.asarray(q_bias, np.float32)
    v_bias = np.asarray(v_bias, np.float32)
    proj_w = np.asarray(proj_w, np.float32)
    proj_b = np.asarray(proj_b, np.float32)

    wqkT = np.ascontiguousarray(qkv_w.T)  # [C, 3C]
    wqkT[:, :C] *= SCALE
    qkb = np.concatenate([q_bias * SCALE, np.zeros(C, np.float32)])
    pb_eff = (proj_b + proj_w @ v_bias).astype(np.float32)
    pwT = np.ascontiguousarray(proj_w.T)
    wqkT_bf = wqkT.astype(ml_dtypes.bfloat16)
    pwT_bf = pwT.astype(ml_dtypes.bfloat16)
    mb = np.where(mask, np.float32(MASK_NEG), np.float32(0.0)).astype(np.float32)

    if "nc" not in _CACHE:
        _CACHE["nc"] = _build_nc()
    nc = _CACHE["nc"]

    in_maps = []
    for b in range(B):
        in_maps.append(
            {
                "xT": np.ascontiguousarray(x[b].T).astype(ml_dtypes.bfloat16),
                "wqkT": wqkT_bf,
                "pwT": pwT_bf,
                "qkb": qkb,
                "mb": np.ascontiguousarray(mb[b]),
                "pb": pb_eff,
            }
        )

    _CACHE["last_in_maps"] = in_maps
    res = run_bass_kernel_spmd(nc, in_maps, list(range(B)))
    out = np.stack([res.results[b]["out"].T for b in range(B)], axis=0)
    return out.astype(np.float32)


if __name__ == "__main__":
    np.random.seed(0)
    x = np.random.randn(B, N, C).astype(np.float32)
    mask = np.random.randint(0, 2, (B, N)) > 0
    qkv_w = (np.random.randn(F3, C) * 0.02).astype(np.float32)
    q_bias = (np.random.randn(C) * 0.02).astype(np.float32)
    v_bias = (np.random.randn(C) * 0.02).astype(np.float32)
    proj_w = (np.random.randn(C, C) * 0.02).astype(np.float32)
    proj_b = (np.random.randn(C) * 0.02).astype(np.float32)
    out = kernel(x, mask, qkv_w, q_bias, v_bias, proj_w, proj_b)
    print(out.shape, out.dtype)
